# revision 5
# baseline (speedup 1.0000x reference)
"""Multi-head attention Bass/Tile kernel for TRN2, 8-core SPMD.

Sharding: core c handles batch b = c//2 and head-group g = c%2 (6 of 12 heads).
Each core computes its 6 heads end-to-end plus a partial output projection
(over its 384 of 768 ctx dims); the host sums the two partials per batch.

v2: single fused PE stream. k-proj first (DMA-prioritized so scores start
~25us in), then a flat (s-block, head-pair) attention loop with a 1-group
exp pipeline carried across pair/block boundaries. v-proj, q-proj of the
next s-block, and the output projection of the previous s-block are woven
into the attention stream to fill PE slack while ScalarE (exp, the ~197us
serial floor) streams continuously. Normalize uses reciprocal_approx_fast
(single DVE op) instead of the iterative reciprocal (3.3us per call).
exp and v are bf16 (halves SBUF traffic, same 1 cyc/row matmul rate).
"""

from contextlib import ExitStack

import numpy as np

import concourse.bass as bass
import concourse.tile as tile
from concourse import bacc, mybir
from concourse._compat import with_exitstack

F32R = mybir.dt.float32r
F32 = mybir.dt.float32
BF16 = mybir.dt.bfloat16
AF = mybir.ActivationFunctionType

B, E, S, H, D = 4, 768, 2048, 12, 64
NH = 6          # heads per core
HD = NH * D     # 384 head-dims per core
NE = E // 128   # 6 e-chunks
NM = HD // 128  # 3 m-chunks (2 heads each)
NT = S // 128   # 16 t-tiles
SBW = 512       # s-block width
NS = S // SBW   # 4 s-blocks
VW = 96         # v_aug width: col 0 = ones (denominator), 32..95 = v-dims


@with_exitstack
def mha_tile(ctx: ExitStack, tc, hs, wq, wk, wv, bq, bk, bv, woT, bo2, outT):
    nc = tc.nc

    persist = ctx.enter_context(tc.tile_pool(name="persist", bufs=1))

    # --- persistent tiles ---
    wk_sb = [persist.tile([128, HD], F32R, name=f"wk{e}") for e in range(NE)]
    wq_sb = [persist.tile([128, HD], F32R, name=f"wq{e}") for e in range(NE)]
    wv_sb = [persist.tile([128, HD], F32R, name=f"wv{e}") for e in range(NE)]
    woT_sb = [persist.tile([128, E], F32R, name=f"wo{f}") for f in range(NM)]
    # hs as separate [128, SBW] tiles per (e, col-block) for fine-grained
    # DMA-arrival dependencies (k-proj of block s waits only on its columns)
    hs_sb = [
        [persist.tile([128, SBW], F32R, name=f"hs{e}_{cb}") for cb in range(NS)]
        for e in range(NE)
    ]
    bq_sb = persist.tile([128, NM], F32, name="bq")
    bk_sb = persist.tile([128, NM], F32, name="bk")
    bv_bc = persist.tile([128, HD], F32, name="bv")
    bo_sb = persist.tile([128, NE], F32, name="bo")

    kT_sb = [persist.tile([128, S], F32R, name=f"kT{m}") for m in range(NM)]
    # qT / ctxT as per-s-block tiles (written/read in disjoint chunks)
    qT_sb = [
        [persist.tile([128, SBW], F32R, name=f"qT{m}_{s}") for s in range(NS)]
        for m in range(NM)
    ]
    ctxT_sb = [
        [persist.tile([128, SBW], F32R, name=f"ctxT{m}_{s}") for s in range(NS)]
        for m in range(NM)
    ]
    v_aug = [persist.tile([128, NH, VW], BF16, name=f"vaug{t}") for t in range(NT)]

    # --- DMA issue order = need order: wk+hs(block0) first, then the rest of
    # hs, then wq (needed at ~21us for q-s0), wv, biases, woT (needed ~70us) ---
    for e in range(NE):
        esl = slice(128 * e, 128 * (e + 1))
        nc.sync.dma_start(wk_sb[e][:], wk[esl, :].bitcast(F32R))
        nc.sync.dma_start(hs_sb[e][0][:], hs[esl, 0:SBW].bitcast(F32R))
    for cb in range(1, NS):
        csl = slice(SBW * cb, SBW * (cb + 1))
        for e in range(NE):
            nc.sync.dma_start(
                hs_sb[e][cb][:], hs[128 * e : 128 * (e + 1), csl].bitcast(F32R)
            )
    for e in range(NE):
        esl = slice(128 * e, 128 * (e + 1))
        nc.sync.dma_start(wq_sb[e][:], wq[esl, :].bitcast(F32R))
    for e in range(NE):
        esl = slice(128 * e, 128 * (e + 1))
        nc.sync.dma_start(wv_sb[e][:], wv[esl, :].bitcast(F32R))
    nc.sync.dma_start(bq_sb[:], bq.rearrange("(m p) -> p m", p=128))
    nc.sync.dma_start(bk_sb[:], bk.rearrange("(m p) -> p m", p=128))
    nc.sync.dma_start(
        bv_bc[:], bass.AP(tensor=bv.tensor, offset=bv.offset, ap=[[0, 128], [1, HD]])
    )
    nc.sync.dma_start(bo_sb[:], bo2.rearrange("(m p) -> p m", p=128))
    for f in range(NM):
        nc.sync.dma_start(woT_sb[f][:], woT[128 * f : 128 * (f + 1), :].bitcast(F32R))

    # v_aug init on DVE (runs during initial DMA wait): zero cols 0..31 of
    # each head slot (col 0 then overwritten with ones; 32..95 written by
    # v-proj)
    for t in range(NT):
        nc.vector.memset(v_aug[t][:, :, 0:32], 0.0)
        nc.vector.memset(v_aug[t][:, :, 0:1], 1.0)

    # --- pools ---
    pssc = ctx.enter_context(tc.tile_pool(name="pssc", bufs=2, space="PSUM"))
    psctx = ctx.enter_context(tc.tile_pool(name="psctx", bufs=1, space="PSUM"))
    ps3 = ctx.enter_context(tc.tile_pool(name="ps3", bufs=2, space="PSUM"))
    expp = ctx.enter_context(tc.tile_pool(name="expp", bufs=2))
    smp = ctx.enter_context(tc.tile_pool(name="smp", bufs=1))
    outp = ctx.enter_context(tc.tile_pool(name="outp", bufs=3))

    # --- emission helpers ---
    def k_proj(m, s):
        msl = slice(128 * m, 128 * (m + 1))
        kp = ps3.tile([128, SBW], F32, tag="o")
        for e in range(NE):
            nc.tensor.matmul(
                kp[:], wk_sb[e][:, msl], hs_sb[e][s][:],
                start=(e == 0), stop=(e == NE - 1),
            )
        nc.vector.tensor_scalar_add(
            out=kT_sb[m][:, SBW * s : SBW * (s + 1)], in0=kp[:],
            scalar1=bk_sb[:, m : m + 1],
        )

    def q_proj(m, s):
        msl = slice(128 * m, 128 * (m + 1))
        qp = ps3.tile([128, SBW], F32, tag="o")
        for e in range(NE):
            nc.tensor.matmul(
                qp[:], wq_sb[e][:, msl], hs_sb[e][s][:],
                start=(e == 0), stop=(e == NE - 1),
            )
        nc.vector.tensor_scalar_add(
            out=qT_sb[m][s][:], in0=qp[:], scalar1=bq_sb[:, m : m + 1]
        )

    def v_proj(t):
        cb, o = t // 4, 128 * (t % 4)
        vp = ps3.tile([128, SBW], F32, tag="o")
        for e in range(NE):
            nc.tensor.matmul(
                vp[:, 0:HD], hs_sb[e][cb][:, o : o + 128], wv_sb[e][:, :],
                start=(e == 0), stop=(e == NE - 1),
            )
        nc.vector.tensor_add(
            out=v_aug[t][:, :, 32 : 32 + D],
            in0=vp[:, 0:HD].rearrange("p (h d) -> p h d", h=NH),
            in1=bv_bc[:].rearrange("p (h d) -> p h d", h=NH),
        )

    def out_proj_chunk(s, et):
        esl = slice(128 * et, 128 * (et + 1))
        op = ps3.tile([128, SBW], F32, tag="o")
        for f in range(NM):
            nc.tensor.matmul(
                op[:], woT_sb[f][:, esl], ctxT_sb[f][s][:],
                start=(f == 0), stop=(f == NM - 1),
            )
        ob = outp.tile([128, SBW], F32, tag="ob")
        nc.vector.tensor_scalar_add(
            out=ob[:], in0=op[:], scalar1=bo_sb[:, et : et + 1]
        )
        nc.sync.dma_start(outT[esl, SBW * s : SBW * (s + 1)], ob[:])

    ITEMS = [(s, p) for s in range(NS) for p in range(NM)]

    ctx_ps = {}  # item idx -> (cpA, cpB)

    def emit_sc_exp(i, g):
        s, p = ITEMS[i]
        tsl = slice(128 * g, 128 * (g + 1))
        scA = pssc.tile([128, SBW], F32, tag="scA")
        scB = pssc.tile([128, SBW], F32, tag="scB")
        nc.tensor.matmul(
            scA[:], kT_sb[p][0:D, tsl], qT_sb[p][s][0:D, :], start=True, stop=True
        )
        nc.tensor.matmul(
            scB[:], kT_sb[p][D:128, tsl], qT_sb[p][s][D:128, :], start=True, stop=True
        )
        exA = expp.tile([128, SBW], BF16, tag="exA")
        exB = expp.tile([128, SBW], BF16, tag="exB")
        nc.scalar.activation(exA[:], scA[:], AF.Exp)
        nc.scalar.activation(exB[:], scB[:], AF.Exp)
        return exA, exB

    def emit_ctx(i, g, exA, exB):
        s, p = ITEMS[i]
        if g == 0:
            cpA = psctx.tile([128, SBW], F32, tag="cA")
            cpB = psctx.tile([128, SBW], F32, tag="cB")
            ctx_ps[i] = (cpA, cpB)
        cpA, cpB = ctx_ps[i]
        st, sp = (g == 0), (g == NT - 1)
        nc.tensor.matmul(
            cpA[0:VW, :], v_aug[g][:, 2 * p, :], exA[:], start=st, stop=sp
        )
        nc.tensor.matmul(
            cpB[0:VW, :], v_aug[g][:, 2 * p + 1, :], exB[:], start=st, stop=sp
        )

    def normalize(i):
        s, p = ITEMS[i]
        cpA, cpB = ctx_ps.pop(i)
        for h, cp in ((2 * p, cpA), (2 * p + 1, cpB)):
            o = D * (h % 2)
            tg = "A" if h % 2 == 0 else "B"
            cu = smp.tile([VW, SBW], F32, tag=f"cu{tg}")
            nc.vector.tensor_copy(cu[:], cp[0:VW, :])
            rec = smp.tile([1, SBW], F32, tag=f"rec{tg}")
            nc.vector.reciprocal_approx_fast(out=rec[0:1, :], in_=cu[0:1, :])
            bcs = smp.tile([VW, SBW], F32, tag=f"bcs{tg}")
            nc.gpsimd.partition_broadcast(bcs[:], rec[0:1, :])
            for q in range(2):  # 32-partition chunks (partition shift rules)
                nc.vector.tensor_mul(
                    out=ctxT_sb[p][s][o + 32 * q : o + 32 * (q + 1), :],
                    in0=cu[32 + 32 * q : 64 + 32 * q, :],
                    in1=bcs[32 + 32 * q : 64 + 32 * q, :],
                )

    # extras[(item, g)] -> list of thunks emitted after sc, before ctx
    extras = {}
    for t in range(NT):  # v-proj woven into item 0 (first ctx use is lag 1)
        extras.setdefault((0, t), []).append(lambda t=t: v_proj(t))
    for s in range(NS - 1):  # q(s+1) during pair (s, 1)
        for m in range(NM):
            extras.setdefault((3 * s + 1, 1 + 4 * m), []).append(
                lambda m=m, s=s: q_proj(m, s + 1)
            )
    for s in range(1, NS):  # out-proj(s-1) during pair (s, 0), after normalize
        for et in range(NE):
            extras.setdefault((3 * s, 1 + 2 * et), []).append(
                lambda s=s, et=et: out_proj_chunk(s - 1, et)
            )

    # --- pre-attention: k-proj (DMA-gated), q(s0) ---
    for s in range(NS):
        for m in range(NM):
            k_proj(m, s)
    for m in range(NM):
        q_proj(m, 0)

    # --- fused attention stream with 1-group exp lag across boundaries ---
    pend = None
    for i in range(len(ITEMS)):
        for g in range(NT):
            ex = emit_sc_exp(i, g)
            for fn in extras.get((i, g), ()):
                fn()
            if pend is not None:
                pi, pg, pA, pB = pend
                emit_ctx(pi, pg, pA, pB)
                if pg == NT - 1:
                    normalize(pi)
            pend = (i, g, ex[0], ex[1])
    pi, pg, pA, pB = pend
    emit_ctx(pi, pg, pA, pB)
    normalize(pi)
    for et in range(NE):
        out_proj_chunk(NS - 1, et)


def build_nc():
    nc = bacc.Bacc("TRN2", target_bir_lowering=False, debug=False)
    hs = nc.dram_tensor("hs", [E, S], F32, kind="ExternalInput")
    wq = nc.dram_tensor("wq", [E, HD], F32, kind="ExternalInput")
    wk = nc.dram_tensor("wk", [E, HD], F32, kind="ExternalInput")
    wv = nc.dram_tensor("wv", [E, HD], F32, kind="ExternalInput")
    bq = nc.dram_tensor("bq", [HD], F32, kind="ExternalInput")
    bk = nc.dram_tensor("bk", [HD], F32, kind="ExternalInput")
    bv = nc.dram_tensor("bv", [HD], F32, kind="ExternalInput")
    woT = nc.dram_tensor("woT", [HD, E], F32, kind="ExternalInput")
    bo2 = nc.dram_tensor("bo2", [E], F32, kind="ExternalInput")
    outT = nc.dram_tensor("outT", [E, S], F32, kind="ExternalOutput")

    with tile.TileContext(nc) as tc:
        mha_tile(
            tc,
            hs[:, :], wq[:, :], wk[:, :], wv[:, :],
            bq[:], bk[:], bv[:],
            woT[:, :], bo2[:], outT[:, :],
        )
    nc.compile()
    return nc


def make_core_inputs(inputs: dict) -> list[dict]:
    """Full inputs -> per-core input maps (core c: batch c//2, head-group c%2)."""
    hsf = np.ascontiguousarray(np.asarray(inputs["hidden_state"], dtype=np.float32))
    Wq = np.asarray(inputs["Wq"], dtype=np.float32)
    Wk = np.asarray(inputs["Wk"], dtype=np.float32)
    Wv = np.asarray(inputs["Wv"], dtype=np.float32)
    Wo = np.asarray(inputs["Wo"], dtype=np.float32)
    bq = np.asarray(inputs["bq"], dtype=np.float32)
    bk = np.asarray(inputs["bk"], dtype=np.float32)
    bv = np.asarray(inputs["bv"], dtype=np.float32)
    bo = np.asarray(inputs["bo"], dtype=np.float32)

    maps = []
    for c in range(8):
        b, g = c // 2, c % 2
        hsl = slice(NH * g, NH * (g + 1))
        fsl = slice(HD * g, HD * (g + 1))
        maps.append(
            {
                "hs": np.ascontiguousarray(hsf[b]),
                "wq": np.ascontiguousarray(Wq[hsl].transpose(1, 0, 2).reshape(E, HD)),
                "wk": np.ascontiguousarray(Wk[hsl].transpose(1, 0, 2).reshape(E, HD)),
                "wv": np.ascontiguousarray(Wv[hsl].transpose(1, 0, 2).reshape(E, HD)),
                "bq": np.ascontiguousarray(bq[hsl].reshape(HD)),
                "bk": np.ascontiguousarray(bk[hsl].reshape(HD)),
                "bv": np.ascontiguousarray(bv[hsl].reshape(HD)),
                "woT": np.ascontiguousarray(Wo[:, fsl].T),
                "bo2": np.ascontiguousarray(bo / 2.0),
            }
        )
    return maps


def combine_outputs(core_outs: list) -> np.ndarray:
    """Per-core outT partials -> full [B, E, S] output."""
    return np.stack(
        [core_outs[2 * b]["outT"] + core_outs[2 * b + 1]["outT"] for b in range(B)]
    ).astype(np.float32)


from concourse.bass_utils import run_bass_kernel_spmd

N_CORES = 8
_NC_CACHE = None


def _get_nc():
    global _NC_CACHE
    if _NC_CACHE is None:
        _NC_CACHE = build_nc()
    return _NC_CACHE


def kernel(**inputs) -> np.ndarray:
    """Full-input entry point: shard across 8 cores, run, unshard."""
    maps = make_core_inputs(inputs)
    nc = _get_nc()
    res = run_bass_kernel_spmd(nc, maps, core_ids=list(range(N_CORES)))
    outs = res.results
    return np.stack(
        [outs[2 * b]["outT"] + outs[2 * b + 1]["outT"] for b in range(B)]
    ).astype(np.float32)


# revision 8
# speedup vs baseline: 1.6785x; 1.6785x over previous
"""Multi-head attention Bass/Tile kernel for TRN2, 8-core SPMD.

Sharding: core c handles batch b = c//2 and head-group g = c%2 (6 of 12 heads).
Each core computes its 6 heads end-to-end plus a partial output projection
(over its 384 of 768 ctx dims); the host sums the two partials per batch.

v2: single fused PE stream. k-proj first (DMA-prioritized so scores start
~25us in), then a flat (s-block, head-pair) attention loop with a 1-group
exp pipeline carried across pair/block boundaries. v-proj, q-proj of the
next s-block, and the output projection of the previous s-block are woven
into the attention stream to fill PE slack while ScalarE (exp, the ~197us
serial floor) streams continuously. Normalize uses reciprocal_approx_fast
(single DVE op) instead of the iterative reciprocal (3.3us per call).
exp and v are bf16 (halves SBUF traffic, same 1 cyc/row matmul rate).
"""

from contextlib import ExitStack

import numpy as np

import concourse.bass as bass
import concourse.tile as tile
from concourse import bacc, mybir
from concourse._compat import with_exitstack

F32R = mybir.dt.float32r
F32 = mybir.dt.float32
BF16 = mybir.dt.bfloat16
AF = mybir.ActivationFunctionType

B, E, S, H, D = 4, 768, 2048, 12, 64
NH = 6          # heads per core
HD = NH * D     # 384 head-dims per core
NE = E // 128   # 6 e-chunks
NM = HD // 128  # 3 m-chunks (2 heads each)
NT = S // 128   # 16 t-tiles
SBW = 512       # s-block width
NS = S // SBW   # 4 s-blocks
VW = 96         # v_aug width: col 0 = ones (denominator), 32..95 = v-dims


@with_exitstack
def mha_tile(ctx: ExitStack, tc, hs, wq, wk, wv, bq, bk, bv, woT, bo2, outT):
    nc = tc.nc

    persist = ctx.enter_context(tc.tile_pool(name="persist", bufs=1))

    # --- persistent tiles ---
    wk_sb = [persist.tile([128, HD], F32R, name=f"wk{e}") for e in range(NE)]
    wq_sb = [persist.tile([128, HD], F32R, name=f"wq{e}") for e in range(NE)]
    wv_sb = [persist.tile([128, HD], F32R, name=f"wv{e}") for e in range(NE)]
    woT_sb = [persist.tile([128, E], F32R, name=f"wo{f}") for f in range(NM)]
    # hs as separate [128, SBW] tiles per (e, col-block) for fine-grained
    # DMA-arrival dependencies (k-proj of block s waits only on its columns)
    hs_sb = [
        [persist.tile([128, SBW], F32R, name=f"hs{e}_{cb}") for cb in range(NS)]
        for e in range(NE)
    ]
    bq_sb = persist.tile([128, NM], F32, name="bq")
    bk_sb = persist.tile([128, NM], F32, name="bk")
    bv_bc = persist.tile([128, HD], F32, name="bv")
    bo_sb = persist.tile([128, NE], F32, name="bo")

    kT_sb = [persist.tile([128, S], F32R, name=f"kT{m}") for m in range(NM)]
    # qT / ctxT as per-s-block tiles (written/read in disjoint chunks)
    qT_sb = [
        [persist.tile([128, SBW], F32R, name=f"qT{m}_{s}") for s in range(NS)]
        for m in range(NM)
    ]
    ctxT_sb = [
        [persist.tile([128, SBW], F32R, name=f"ctxT{m}_{s}") for s in range(NS)]
        for m in range(NM)
    ]
    v_aug = [persist.tile([128, NH, VW], BF16, name=f"vaug{t}") for t in range(NT)]

    # --- DMA issue order = need order: wk + hs(block0), wv, wq, remaining hs
    # blocks, biases, woT (needed ~70us). PE streams k/v/q0 behind arrivals. ---
    for e in range(NE):
        esl = slice(128 * e, 128 * (e + 1))
        nc.sync.dma_start(wk_sb[e][:], wk[esl, :].bitcast(F32R))
        nc.sync.dma_start(hs_sb[e][0][:], hs[esl, 0:SBW].bitcast(F32R))
    for e in range(NE):
        esl = slice(128 * e, 128 * (e + 1))
        nc.sync.dma_start(wv_sb[e][:], wv[esl, :].bitcast(F32R))
    for e in range(NE):
        esl = slice(128 * e, 128 * (e + 1))
        nc.sync.dma_start(wq_sb[e][:], wq[esl, :].bitcast(F32R))
    for cb in range(1, NS):
        csl = slice(SBW * cb, SBW * (cb + 1))
        for e in range(NE):
            nc.sync.dma_start(
                hs_sb[e][cb][:], hs[128 * e : 128 * (e + 1), csl].bitcast(F32R)
            )
    nc.sync.dma_start(bq_sb[:], bq.rearrange("(m p) -> p m", p=128))
    nc.sync.dma_start(bk_sb[:], bk.rearrange("(m p) -> p m", p=128))
    nc.sync.dma_start(
        bv_bc[:], bass.AP(tensor=bv.tensor, offset=bv.offset, ap=[[0, 128], [1, HD]])
    )
    nc.sync.dma_start(bo_sb[:], bo2.rearrange("(m p) -> p m", p=128))
    for f in range(NM):
        nc.sync.dma_start(woT_sb[f][:], woT[128 * f : 128 * (f + 1), :].bitcast(F32R))

    # v_aug init on DVE (runs during initial DMA wait): zero cols 0..31 of
    # each head slot (col 0 then overwritten with ones; 32..95 written by
    # v-proj)
    for t in range(NT):
        nc.vector.memset(v_aug[t][:, :, 0:32], 0.0)
        nc.vector.memset(v_aug[t][:, :, 0:1], 1.0)

    # --- pools ---
    pssc = ctx.enter_context(tc.tile_pool(name="pssc", bufs=2, space="PSUM"))
    psctx = ctx.enter_context(tc.tile_pool(name="psctx", bufs=1, space="PSUM"))
    ps3 = ctx.enter_context(tc.tile_pool(name="ps3", bufs=2, space="PSUM"))
    expp = ctx.enter_context(tc.tile_pool(name="expp", bufs=2))
    smp = ctx.enter_context(tc.tile_pool(name="smp", bufs=1))
    outp = ctx.enter_context(tc.tile_pool(name="outp", bufs=3))

    # --- emission helpers ---
    def k_proj(m, s):
        msl = slice(128 * m, 128 * (m + 1))
        kp = ps3.tile([128, SBW], F32, tag="o")
        for e in range(NE):
            nc.tensor.matmul(
                kp[:], wk_sb[e][:, msl], hs_sb[e][s][:],
                start=(e == 0), stop=(e == NE - 1),
            )
        nc.vector.tensor_scalar_add(
            out=kT_sb[m][:, SBW * s : SBW * (s + 1)], in0=kp[:],
            scalar1=bk_sb[:, m : m + 1],
        )

    def q_proj(m, s):
        msl = slice(128 * m, 128 * (m + 1))
        qp = ps3.tile([128, SBW], F32, tag="o")
        for e in range(NE):
            nc.tensor.matmul(
                qp[:], wq_sb[e][:, msl], hs_sb[e][s][:],
                start=(e == 0), stop=(e == NE - 1),
            )
        nc.vector.tensor_scalar_add(
            out=qT_sb[m][s][:], in0=qp[:], scalar1=bq_sb[:, m : m + 1]
        )

    def v_proj(t):
        cb, o = t // 4, 128 * (t % 4)
        vp = ps3.tile([128, SBW], F32, tag="o")
        for e in range(NE):
            nc.tensor.matmul(
                vp[:, 0:HD], hs_sb[e][cb][:, o : o + 128], wv_sb[e][:, :],
                start=(e == 0), stop=(e == NE - 1),
            )
        nc.vector.tensor_add(
            out=v_aug[t][:, :, 32 : 32 + D],
            in0=vp[:, 0:HD].rearrange("p (h d) -> p h d", h=NH),
            in1=bv_bc[:].rearrange("p (h d) -> p h d", h=NH),
        )

    def out_proj_chunk(s, et):
        esl = slice(128 * et, 128 * (et + 1))
        op = ps3.tile([128, SBW], F32, tag="o")
        for f in range(NM):
            nc.tensor.matmul(
                op[:], woT_sb[f][:, esl], ctxT_sb[f][s][:],
                start=(f == 0), stop=(f == NM - 1),
            )
        ob = outp.tile([128, SBW], F32, tag="ob")
        nc.vector.tensor_scalar_add(
            out=ob[:], in0=op[:], scalar1=bo_sb[:, et : et + 1]
        )
        nc.sync.dma_start(outT[esl, SBW * s : SBW * (s + 1)], ob[:])

    ITEMS = [(s, p) for s in range(NS) for p in range(NM)]

    ctx_ps = {}  # item idx -> (cpA, cpB)

    def emit_sc_exp(i, g):
        # one [128, 1024] PSUM tile holds both head halves -> single wide exp
        s, p = ITEMS[i]
        tsl = slice(128 * g, 128 * (g + 1))
        sc = pssc.tile([128, 2 * SBW], F32, tag="sc")
        nc.tensor.matmul(
            sc[:, 0:SBW], kT_sb[p][0:D, tsl], qT_sb[p][s][0:D, :],
            start=True, stop=True,
        )
        nc.tensor.matmul(
            sc[:, SBW : 2 * SBW], kT_sb[p][D:128, tsl], qT_sb[p][s][D:128, :],
            start=True, stop=True,
        )
        ex = expp.tile([128, 2 * SBW], BF16, tag="ex")
        nc.scalar.activation(ex[:], sc[:], AF.Exp)
        return ex

    def emit_ctx(i, g, ex):
        s, p = ITEMS[i]
        if g == 0:
            cpA = psctx.tile([128, SBW], F32, tag="cA")
            cpB = psctx.tile([128, SBW], F32, tag="cB")
            ctx_ps[i] = (cpA, cpB)
        cpA, cpB = ctx_ps[i]
        st, sp = (g == 0), (g == NT - 1)
        nc.tensor.matmul(
            cpA[0:VW, :], v_aug[g][:, 2 * p, :], ex[:, 0:SBW], start=st, stop=sp
        )
        nc.tensor.matmul(
            cpB[0:VW, :], v_aug[g][:, 2 * p + 1, :], ex[:, SBW : 2 * SBW],
            start=st, stop=sp,
        )

    def normalize(i):
        s, p = ITEMS[i]
        cpA, cpB = ctx_ps.pop(i)
        for h, cp in ((2 * p, cpA), (2 * p + 1, cpB)):
            o = D * (h % 2)
            tg = "A" if h % 2 == 0 else "B"
            cu = smp.tile([VW, SBW], F32, tag=f"cu{tg}")
            nc.vector.tensor_copy(cu[:], cp[0:VW, :])
            rec = smp.tile([1, SBW], F32, tag=f"rec{tg}")
            nc.vector.reciprocal_approx_fast(out=rec[0:1, :], in_=cu[0:1, :])
            bcs = smp.tile([VW, SBW], F32, tag=f"bcs{tg}")
            nc.gpsimd.partition_broadcast(bcs[:], rec[0:1, :])
            for q in range(2):  # 32-partition chunks (partition shift rules)
                nc.vector.tensor_mul(
                    out=ctxT_sb[p][s][o + 32 * q : o + 32 * (q + 1), :],
                    in0=cu[32 + 32 * q : 64 + 32 * q, :],
                    in1=bcs[32 + 32 * q : 64 + 32 * q, :],
                )

    # extras[(item, g)] -> list of thunks emitted after sc, before ctx.
    # These keep the PE slightly ahead of ScalarE (the exp floor) everywhere
    # so the tensor engine never idles and stays at full p-state.
    extras = {}
    for s in range(NS - 1):  # q(s+1) during pair (s, 1)
        for m in range(NM):
            extras.setdefault((3 * s + 1, 2 + 5 * m), []).append(
                lambda m=m, s=s: q_proj(m, s + 1)
            )
    for s in range(1, NS):  # out-proj(s-1) during pair (s, 0), after normalize
        for et in range(NE):
            extras.setdefault((3 * s, 1 + 2 * et), []).append(
                lambda s=s, et=et: out_proj_chunk(s - 1, et)
            )

    # --- pre-attention: k/v interleaved behind hs column-block DMA arrivals,
    # q(s0) as soon as wq lands; one continuous PE stream, no stalls ---
    for m in range(NM):
        k_proj(m, 0)
    for t in range(4):
        v_proj(t)
    for m in range(NM):
        q_proj(m, 0)
    for cb in range(1, NS):
        for m in range(NM):
            k_proj(m, cb)
        for t in range(4 * cb, 4 * cb + 4):
            v_proj(t)

    # --- fused attention stream with 1-group exp lag across boundaries ---
    pend = None
    for i in range(len(ITEMS)):
        for g in range(NT):
            ex = emit_sc_exp(i, g)
            for fn in extras.get((i, g), ()):
                fn()
            if pend is not None:
                pi, pg, pex = pend
                emit_ctx(pi, pg, pex)
                if pg == NT - 1:
                    normalize(pi)
            pend = (i, g, ex)
    pi, pg, pex = pend
    emit_ctx(pi, pg, pex)
    normalize(pi)
    for et in range(NE):
        out_proj_chunk(NS - 1, et)


def build_nc():
    nc = bacc.Bacc("TRN2", target_bir_lowering=False, debug=False)
    hs = nc.dram_tensor("hs", [E, S], F32, kind="ExternalInput")
    wq = nc.dram_tensor("wq", [E, HD], F32, kind="ExternalInput")
    wk = nc.dram_tensor("wk", [E, HD], F32, kind="ExternalInput")
    wv = nc.dram_tensor("wv", [E, HD], F32, kind="ExternalInput")
    bq = nc.dram_tensor("bq", [HD], F32, kind="ExternalInput")
    bk = nc.dram_tensor("bk", [HD], F32, kind="ExternalInput")
    bv = nc.dram_tensor("bv", [HD], F32, kind="ExternalInput")
    woT = nc.dram_tensor("woT", [HD, E], F32, kind="ExternalInput")
    bo2 = nc.dram_tensor("bo2", [E], F32, kind="ExternalInput")
    outT = nc.dram_tensor("outT", [E, S], F32, kind="ExternalOutput")

    with tile.TileContext(nc) as tc:
        mha_tile(
            tc,
            hs[:, :], wq[:, :], wk[:, :], wv[:, :],
            bq[:], bk[:], bv[:],
            woT[:, :], bo2[:], outT[:, :],
        )
    nc.compile()
    return nc


def make_core_inputs(inputs: dict) -> list[dict]:
    """Full inputs -> per-core input maps (core c: batch c//2, head-group c%2)."""
    hsf = np.ascontiguousarray(np.asarray(inputs["hidden_state"], dtype=np.float32))
    Wq = np.asarray(inputs["Wq"], dtype=np.float32)
    Wk = np.asarray(inputs["Wk"], dtype=np.float32)
    Wv = np.asarray(inputs["Wv"], dtype=np.float32)
    Wo = np.asarray(inputs["Wo"], dtype=np.float32)
    bq = np.asarray(inputs["bq"], dtype=np.float32)
    bk = np.asarray(inputs["bk"], dtype=np.float32)
    bv = np.asarray(inputs["bv"], dtype=np.float32)
    bo = np.asarray(inputs["bo"], dtype=np.float32)

    maps = []
    for c in range(8):
        b, g = c // 2, c % 2
        hsl = slice(NH * g, NH * (g + 1))
        fsl = slice(HD * g, HD * (g + 1))
        maps.append(
            {
                "hs": np.ascontiguousarray(hsf[b]),
                "wq": np.ascontiguousarray(Wq[hsl].transpose(1, 0, 2).reshape(E, HD)),
                "wk": np.ascontiguousarray(Wk[hsl].transpose(1, 0, 2).reshape(E, HD)),
                "wv": np.ascontiguousarray(Wv[hsl].transpose(1, 0, 2).reshape(E, HD)),
                "bq": np.ascontiguousarray(bq[hsl].reshape(HD)),
                "bk": np.ascontiguousarray(bk[hsl].reshape(HD)),
                "bv": np.ascontiguousarray(bv[hsl].reshape(HD)),
                "woT": np.ascontiguousarray(Wo[:, fsl].T),
                "bo2": np.ascontiguousarray(bo / 2.0),
            }
        )
    return maps


def combine_outputs(core_outs: list) -> np.ndarray:
    """Per-core outT partials -> full [B, E, S] output."""
    return np.stack(
        [core_outs[2 * b]["outT"] + core_outs[2 * b + 1]["outT"] for b in range(B)]
    ).astype(np.float32)


from concourse.bass_utils import run_bass_kernel_spmd

N_CORES = 8
_NC_CACHE = None


def _get_nc():
    global _NC_CACHE
    if _NC_CACHE is None:
        _NC_CACHE = build_nc()
    return _NC_CACHE


def kernel(**inputs) -> np.ndarray:
    """Full-input entry point: shard across 8 cores, run, unshard."""
    maps = make_core_inputs(inputs)
    nc = _get_nc()
    res = run_bass_kernel_spmd(nc, maps, core_ids=list(range(N_CORES)))
    outs = res.results
    return np.stack(
        [outs[2 * b]["outT"] + outs[2 * b + 1]["outT"] for b in range(B)]
    ).astype(np.float32)


# revision 12
# speedup vs baseline: 1.7182x; 1.0236x over previous
"""Multi-head attention Bass/Tile kernel for TRN2, 8-core SPMD.

Sharding: core c handles batch b = c//2 and head-group g = c%2 (6 of 12 heads).
Each core computes its 6 heads end-to-end plus a partial output projection
(over its 384 of 768 ctx dims); the host sums the two partials per batch.

v2: single fused PE stream. k-proj first (DMA-prioritized so scores start
~25us in), then a flat (s-block, head-pair) attention loop with a 1-group
exp pipeline carried across pair/block boundaries. v-proj, q-proj of the
next s-block, and the output projection of the previous s-block are woven
into the attention stream to fill PE slack while ScalarE (exp, the ~197us
serial floor) streams continuously. Normalize uses reciprocal_approx_fast
(single DVE op) instead of the iterative reciprocal (3.3us per call).
exp and v are bf16 (halves SBUF traffic, same 1 cyc/row matmul rate).
"""

from contextlib import ExitStack

import numpy as np

import concourse.bass as bass
import concourse.tile as tile
from concourse import bacc, mybir
from concourse._compat import with_exitstack

F32R = mybir.dt.float32r
F32 = mybir.dt.float32
BF16 = mybir.dt.bfloat16
AF = mybir.ActivationFunctionType

B, E, S, H, D = 4, 768, 2048, 12, 64
NH = 6          # heads per core
HD = NH * D     # 384 head-dims per core
NE = E // 128   # 6 e-chunks
NM = HD // 128  # 3 m-chunks (2 heads each)
NT = S // 128   # 16 t-tiles
SBW = 512       # s-block width
NS = S // SBW   # 4 s-blocks
VW = 96         # v_aug width: col 0 = ones (denominator), 32..95 = v-dims


@with_exitstack
def mha_tile(ctx: ExitStack, tc, hs, wq, wk, wv, bq, bk, bv, woT, bo2, outT):
    nc = tc.nc

    persist = ctx.enter_context(tc.tile_pool(name="persist", bufs=1))

    # --- persistent tiles ---
    wk_sb = [persist.tile([128, HD], F32R, name=f"wk{e}") for e in range(NE)]
    wq_sb = [persist.tile([128, HD], F32R, name=f"wq{e}") for e in range(NE)]
    wv_sb = [persist.tile([128, HD], F32R, name=f"wv{e}") for e in range(NE)]
    woT_sb = [persist.tile([128, E], F32R, name=f"wo{f}") for f in range(NM)]
    # hs as separate [128, SBW] tiles per (e, col-block) for fine-grained
    # DMA-arrival dependencies (k-proj of block s waits only on its columns)
    hs_sb = [
        [persist.tile([128, SBW], F32R, name=f"hs{e}_{cb}") for cb in range(NS)]
        for e in range(NE)
    ]
    bq_sb = persist.tile([128, NM], F32, name="bq")
    bk_sb = persist.tile([128, NM], F32, name="bk")
    bv_bc = persist.tile([128, HD], F32, name="bv")
    bo_sb = persist.tile([128, NE], F32, name="bo")

    kT_sb = [persist.tile([128, S], F32R, name=f"kT{m}") for m in range(NM)]
    # qT / ctxT as per-s-block tiles (written/read in disjoint chunks)
    qT_sb = [
        [persist.tile([128, SBW], F32R, name=f"qT{m}_{s}") for s in range(NS)]
        for m in range(NM)
    ]
    ctxT_sb = [
        [persist.tile([128, SBW], F32R, name=f"ctxT{m}_{s}") for s in range(NS)]
        for m in range(NM)
    ]
    v_aug = [persist.tile([128, NH, VW], BF16, name=f"vaug{t}") for t in range(NT)]

    # --- DMA issue order = need order: wk + hs(block0), wv, wq, remaining hs
    # blocks, biases, woT (needed ~70us). PE streams k/v/q0 behind arrivals. ---
    for e in range(NE):
        esl = slice(128 * e, 128 * (e + 1))
        nc.sync.dma_start(wk_sb[e][:], wk[esl, :].bitcast(F32R))
        nc.sync.dma_start(hs_sb[e][0][:], hs[esl, 0:SBW].bitcast(F32R))
    for cb in range(1, NS):
        csl = slice(SBW * cb, SBW * (cb + 1))
        for e in range(NE):
            nc.sync.dma_start(
                hs_sb[e][cb][:], hs[128 * e : 128 * (e + 1), csl].bitcast(F32R)
            )
    for e in range(NE):
        esl = slice(128 * e, 128 * (e + 1))
        nc.sync.dma_start(wq_sb[e][:], wq[esl, :].bitcast(F32R))
    for e in range(NE):
        esl = slice(128 * e, 128 * (e + 1))
        nc.sync.dma_start(wv_sb[e][:], wv[esl, :].bitcast(F32R))
    nc.sync.dma_start(bq_sb[:], bq.rearrange("(m p) -> p m", p=128))
    nc.sync.dma_start(bk_sb[:], bk.rearrange("(m p) -> p m", p=128))
    nc.sync.dma_start(
        bv_bc[:], bass.AP(tensor=bv.tensor, offset=bv.offset, ap=[[0, 128], [1, HD]])
    )
    nc.sync.dma_start(bo_sb[:], bo2.rearrange("(m p) -> p m", p=128))
    for f in range(NM):
        nc.sync.dma_start(woT_sb[f][:], woT[128 * f : 128 * (f + 1), :].bitcast(F32R))

    # v_aug init on DVE (runs during initial DMA wait): zero cols 0..31 of
    # each head slot (col 0 then overwritten with ones; 32..95 written by
    # v-proj)
    for t in range(NT):
        nc.vector.memset(v_aug[t][:, :, 0:32], 0.0)
        nc.vector.memset(v_aug[t][:, :, 0:1], 1.0)

    # --- pools ---
    pssc = ctx.enter_context(tc.tile_pool(name="pssc", bufs=2, space="PSUM"))
    psctx = ctx.enter_context(tc.tile_pool(name="psctx", bufs=1, space="PSUM"))
    ps3 = ctx.enter_context(tc.tile_pool(name="ps3", bufs=2, space="PSUM"))
    expp = ctx.enter_context(tc.tile_pool(name="expp", bufs=3))
    smp = ctx.enter_context(tc.tile_pool(name="smp", bufs=1))
    outp = ctx.enter_context(tc.tile_pool(name="outp", bufs=3))

    # --- emission helpers ---
    def k_proj(m, s):
        msl = slice(128 * m, 128 * (m + 1))
        kp = ps3.tile([128, SBW], F32, tag="o")
        for e in range(NE):
            nc.tensor.matmul(
                kp[:], wk_sb[e][:, msl], hs_sb[e][s][:],
                start=(e == 0), stop=(e == NE - 1),
            )
        nc.vector.tensor_scalar_add(
            out=kT_sb[m][:, SBW * s : SBW * (s + 1)], in0=kp[:],
            scalar1=bk_sb[:, m : m + 1],
        )

    def q_proj(m, s):
        msl = slice(128 * m, 128 * (m + 1))
        qp = ps3.tile([128, SBW], F32, tag="o")
        for e in range(NE):
            nc.tensor.matmul(
                qp[:], wq_sb[e][:, msl], hs_sb[e][s][:],
                start=(e == 0), stop=(e == NE - 1),
            )
        nc.vector.tensor_scalar_add(
            out=qT_sb[m][s][:], in0=qp[:], scalar1=bq_sb[:, m : m + 1]
        )

    def v_proj(t):
        cb, o = t // 4, 128 * (t % 4)
        vp = ps3.tile([128, SBW], F32, tag="o")
        for e in range(NE):
            nc.tensor.matmul(
                vp[:, 0:HD], hs_sb[e][cb][:, o : o + 128], wv_sb[e][:, :],
                start=(e == 0), stop=(e == NE - 1),
            )
        nc.vector.tensor_add(
            out=v_aug[t][:, :, 32 : 32 + D],
            in0=vp[:, 0:HD].rearrange("p (h d) -> p h d", h=NH),
            in1=bv_bc[:].rearrange("p (h d) -> p h d", h=NH),
        )

    def out_proj_chunk(s, et):
        esl = slice(128 * et, 128 * (et + 1))
        op = ps3.tile([128, SBW], F32, tag="o")
        for f in range(NM):
            nc.tensor.matmul(
                op[:], woT_sb[f][:, esl], ctxT_sb[f][s][:],
                start=(f == 0), stop=(f == NM - 1),
            )
        ob = outp.tile([128, SBW], F32, tag="ob")
        nc.vector.tensor_scalar_add(
            out=ob[:], in0=op[:], scalar1=bo_sb[:, et : et + 1]
        )
        nc.sync.dma_start(outT[esl, SBW * s : SBW * (s + 1)], ob[:])

    ITEMS = [(s, p) for s in range(NS) for p in range(NM)]

    ctx_ps = {}  # item idx -> (cpA, cpB)

    def emit_sc_exp(i, g):
        # one [128, 1024] PSUM tile holds both head halves -> single wide exp
        s, p = ITEMS[i]
        tsl = slice(128 * g, 128 * (g + 1))
        sc = pssc.tile([128, 2 * SBW], F32, tag="sc")
        nc.tensor.matmul(
            sc[:, 0:SBW], kT_sb[p][0:D, tsl], qT_sb[p][s][0:D, :],
            start=True, stop=True,
        )
        nc.tensor.matmul(
            sc[:, SBW : 2 * SBW], kT_sb[p][D:128, tsl], qT_sb[p][s][D:128, :],
            start=True, stop=True,
        )
        ex = expp.tile([128, 2 * SBW], BF16, tag="ex")
        nc.scalar.activation(ex[:], sc[:], AF.Exp)
        return ex

    def emit_ctx(i, g, ex):
        s, p = ITEMS[i]
        if g == 0:
            cpA = psctx.tile([128, SBW], F32, tag="cA")
            cpB = psctx.tile([128, SBW], F32, tag="cB")
            ctx_ps[i] = (cpA, cpB)
        cpA, cpB = ctx_ps[i]
        st, sp = (g == 0), (g == NT - 1)
        nc.tensor.matmul(
            cpA[0:VW, :], v_aug[g][:, 2 * p, :], ex[:, 0:SBW], start=st, stop=sp
        )
        nc.tensor.matmul(
            cpB[0:VW, :], v_aug[g][:, 2 * p + 1, :], ex[:, SBW : 2 * SBW],
            start=st, stop=sp,
        )

    def normalize(i):
        s, p = ITEMS[i]
        cpA, cpB = ctx_ps.pop(i)
        for h, cp in ((2 * p, cpA), (2 * p + 1, cpB)):
            o = D * (h % 2)
            tg = "A" if h % 2 == 0 else "B"
            cu = smp.tile([VW, SBW], F32, tag=f"cu{tg}")
            nc.vector.tensor_copy(cu[:], cp[0:VW, :])
            rec = smp.tile([1, SBW], F32, tag=f"rec{tg}")
            nc.vector.reciprocal_approx_fast(out=rec[0:1, :], in_=cu[0:1, :])
            bcs = smp.tile([VW, SBW], F32, tag=f"bcs{tg}")
            nc.gpsimd.partition_broadcast(bcs[:], rec[0:1, :])
            for q in range(2):  # 32-partition chunks (partition shift rules)
                nc.vector.tensor_mul(
                    out=ctxT_sb[p][s][o + 32 * q : o + 32 * (q + 1), :],
                    in0=cu[32 + 32 * q : 64 + 32 * q, :],
                    in1=bcs[32 + 32 * q : 64 + 32 * q, :],
                )

    # extras[(item, g)] -> list of thunks emitted after sc, before ctx.
    # These keep the PE slightly ahead of ScalarE (the exp floor) everywhere
    # so the tensor engine never idles and stays at full p-state.
    extras = {}
    for t in range(NT):  # v-proj woven into item 0 (ctx consumes at lag 2)
        extras.setdefault((0, t), []).append(lambda t=t: v_proj(t))
    for s in range(NS - 1):  # q(s+1) during pair (s, 1)
        for m in range(NM):
            extras.setdefault((3 * s + 1, 2 + 5 * m), []).append(
                lambda m=m, s=s: q_proj(m, s + 1)
            )
    # out-proj(s-1) during pair (s, 0): normalize of block s-1's last pair is
    # emitted at unit (3s, 1) under the lag-2 pipeline, so start at g=2
    for s in range(1, NS):
        for et in range(NE):
            extras.setdefault((3 * s, 2 + 2 * et), []).append(
                lambda s=s, et=et: out_proj_chunk(s - 1, et)
            )

    # --- pre-attention: k streams behind hs column-block DMA arrivals, then
    # q(s0); v is woven into the attention stream (item 0) ---
    for cb in range(NS):
        for m in range(NM):
            k_proj(m, cb)
    for m in range(NM):
        q_proj(m, 0)

    # --- fused attention stream with 2-group exp lag across boundaries ---
    from collections import deque

    pend = deque()
    units = [(i, g) for i in range(len(ITEMS)) for g in range(NT)]

    def drain_one():
        pi, pg, pex = pend.popleft()
        emit_ctx(pi, pg, pex)
        if pg == NT - 1:
            normalize(pi)

    for i, g in units:
        ex = emit_sc_exp(i, g)
        for fn in extras.get((i, g), ()):
            fn()
        if len(pend) == 2:
            drain_one()
        pend.append((i, g, ex))
    while pend:
        drain_one()
    for et in range(NE):
        out_proj_chunk(NS - 1, et)


def build_nc():
    nc = bacc.Bacc("TRN2", target_bir_lowering=False, debug=False)
    hs = nc.dram_tensor("hs", [E, S], F32, kind="ExternalInput")
    wq = nc.dram_tensor("wq", [E, HD], F32, kind="ExternalInput")
    wk = nc.dram_tensor("wk", [E, HD], F32, kind="ExternalInput")
    wv = nc.dram_tensor("wv", [E, HD], F32, kind="ExternalInput")
    bq = nc.dram_tensor("bq", [HD], F32, kind="ExternalInput")
    bk = nc.dram_tensor("bk", [HD], F32, kind="ExternalInput")
    bv = nc.dram_tensor("bv", [HD], F32, kind="ExternalInput")
    woT = nc.dram_tensor("woT", [HD, E], F32, kind="ExternalInput")
    bo2 = nc.dram_tensor("bo2", [E], F32, kind="ExternalInput")
    outT = nc.dram_tensor("outT", [E, S], F32, kind="ExternalOutput")

    with tile.TileContext(nc) as tc:
        mha_tile(
            tc,
            hs[:, :], wq[:, :], wk[:, :], wv[:, :],
            bq[:], bk[:], bv[:],
            woT[:, :], bo2[:], outT[:, :],
        )
    nc.compile()
    return nc


def make_core_inputs(inputs: dict) -> list[dict]:
    """Full inputs -> per-core input maps (core c: batch c//2, head-group c%2)."""
    hsf = np.ascontiguousarray(np.asarray(inputs["hidden_state"], dtype=np.float32))
    Wq = np.asarray(inputs["Wq"], dtype=np.float32)
    Wk = np.asarray(inputs["Wk"], dtype=np.float32)
    Wv = np.asarray(inputs["Wv"], dtype=np.float32)
    Wo = np.asarray(inputs["Wo"], dtype=np.float32)
    bq = np.asarray(inputs["bq"], dtype=np.float32)
    bk = np.asarray(inputs["bk"], dtype=np.float32)
    bv = np.asarray(inputs["bv"], dtype=np.float32)
    bo = np.asarray(inputs["bo"], dtype=np.float32)

    maps = []
    for c in range(8):
        b, g = c // 2, c % 2
        hsl = slice(NH * g, NH * (g + 1))
        fsl = slice(HD * g, HD * (g + 1))
        maps.append(
            {
                "hs": np.ascontiguousarray(hsf[b]),
                "wq": np.ascontiguousarray(Wq[hsl].transpose(1, 0, 2).reshape(E, HD)),
                "wk": np.ascontiguousarray(Wk[hsl].transpose(1, 0, 2).reshape(E, HD)),
                "wv": np.ascontiguousarray(Wv[hsl].transpose(1, 0, 2).reshape(E, HD)),
                "bq": np.ascontiguousarray(bq[hsl].reshape(HD)),
                "bk": np.ascontiguousarray(bk[hsl].reshape(HD)),
                "bv": np.ascontiguousarray(bv[hsl].reshape(HD)),
                "woT": np.ascontiguousarray(Wo[:, fsl].T),
                "bo2": np.ascontiguousarray(bo / 2.0),
            }
        )
    return maps


def combine_outputs(core_outs: list) -> np.ndarray:
    """Per-core outT partials -> full [B, E, S] output."""
    return np.stack(
        [core_outs[2 * b]["outT"] + core_outs[2 * b + 1]["outT"] for b in range(B)]
    ).astype(np.float32)


from concourse.bass_utils import run_bass_kernel_spmd

N_CORES = 8
_NC_CACHE = None


def _get_nc():
    global _NC_CACHE
    if _NC_CACHE is None:
        _NC_CACHE = build_nc()
    return _NC_CACHE


def kernel(**inputs) -> np.ndarray:
    """Full-input entry point: shard across 8 cores, run, unshard."""
    maps = make_core_inputs(inputs)
    nc = _get_nc()
    res = run_bass_kernel_spmd(nc, maps, core_ids=list(range(N_CORES)))
    outs = res.results
    return np.stack(
        [outs[2 * b]["outT"] + outs[2 * b + 1]["outT"] for b in range(B)]
    ).astype(np.float32)


# revision 14
# speedup vs baseline: 1.7639x; 1.0266x over previous
"""Multi-head attention Bass/Tile kernel for TRN2, 8-core SPMD.

Sharding: core c handles batch b = c//2 and head-group g = c%2 (6 of 12 heads).
Each core computes its 6 heads end-to-end plus a partial output projection
(over its 384 of 768 ctx dims); the host sums the two partials per batch.

v6 design notes (all calibrated against perfetto traces):
- ScalarE exp is a ~200us serial floor; PE (all matmuls) is ~260us busy.
  The kernel is one fused PE stream ordered so exp starts ~15us in and both
  engines stay saturated: minimal prefix (k-chunk m=0/block0 + q m=0/s0),
  then a flat (s-block, head-pair) attention loop with all other projections
  woven in as "extras" at units where their outputs are first needed.
- 2-group exp pipeline lag: ctx(g-2) is emitted after sc(g), so the PE never
  waits on the exp latency (PE p-state: any stall drops the tensor clock
  2-3.7x; keeping it saturated is worth more than any local reordering).
- DMA queues are descriptor-bound (~78ns per row-descriptor): Wq|Wk|Wv are
  packed host-side into one [E, 1152] tensor (4.5KB rows), hs col-blocks 1-3
  load as [128, 1536] tiles (6KB rows) — inputs land by ~15us.
- PSUM accumulation chains serialize ~50-90ns per link; independent chains
  are emitted pairwise interleaved to hide it.
- Normalize: reciprocal_approx_fast (single DVE op, ~18 bits) + gpsimd
  partition_broadcast; exp/v_aug in bf16 (same PE rate, half the SBUF).
"""

from contextlib import ExitStack

import numpy as np

import concourse.bass as bass
import concourse.tile as tile
from concourse import bacc, mybir
from concourse._compat import with_exitstack

F32R = mybir.dt.float32r
F32 = mybir.dt.float32
BF16 = mybir.dt.bfloat16
AF = mybir.ActivationFunctionType

B, E, S, H, D = 4, 768, 2048, 12, 64
NH = 6          # heads per core
HD = NH * D     # 384 head-dims per core
NE = E // 128   # 6 e-chunks
NM = HD // 128  # 3 m-chunks (2 heads each)
NT = S // 128   # 16 t-tiles
SBW = 512       # s-block width
NS = S // SBW   # 4 s-blocks
VW = 96         # v_aug width: col 0 = ones (denominator), 32..95 = v-dims
WQO, WKO, WVO = 0, HD, 2 * HD  # column offsets in packed wqkv


@with_exitstack
def mha_tile(ctx: ExitStack, tc, hs, wqkv, bq, bk, bv, woT, bo2, outT):
    nc = tc.nc

    persist = ctx.enter_context(tc.tile_pool(name="persist", bufs=1))

    # --- persistent tiles ---
    wqkv_sb = [persist.tile([128, 3 * HD], F32R, name=f"wqkv{e}") for e in range(NE)]
    woT_sb = [persist.tile([128, E], F32R, name=f"wo{f}") for f in range(NM)]
    hs0_sb = [persist.tile([128, SBW], F32R, name=f"hs0_{e}") for e in range(NE)]
    hs123_sb = [persist.tile([128, 3 * SBW], F32R, name=f"hs123_{e}") for e in range(NE)]
    bq_sb = persist.tile([128, NM], F32, name="bq")
    bk_sb = persist.tile([128, NM], F32, name="bk")
    bv_bc = persist.tile([128, HD], F32, name="bv")
    bo_sb = persist.tile([128, NE], F32, name="bo")

    kT_sb = [persist.tile([128, S], F32R, name=f"kT{m}") for m in range(NM)]
    qT_sb = [
        [persist.tile([128, SBW], F32R, name=f"qT{m}_{s}") for s in range(NS)]
        for m in range(NM)
    ]
    ctxT_sb = [
        [persist.tile([128, SBW], F32R, name=f"ctxT{m}_{s}") for s in range(NS)]
        for m in range(NM)
    ]
    v_aug = [persist.tile([128, NH, VW], BF16, name=f"vaug{t}") for t in range(NT)]

    def hs_at(e, s):  # moving [128, 512] for k/q of s-block s
        return hs0_sb[e][:] if s == 0 else hs123_sb[e][:, SBW * (s - 1) : SBW * s]

    def hs_tile_at(e, t):  # stationary [128, 128] for v of t-tile t
        if t < 4:
            return hs0_sb[e][:, 128 * (t % 4) : 128 * (t % 4 + 1)]
        o = 128 * (t - 4)
        return hs123_sb[e][:, o : o + 128]

    # --- DMA issue order = need order ---
    for e in range(NE):
        esl = slice(128 * e, 128 * (e + 1))
        nc.sync.dma_start(wqkv_sb[e][:], wqkv[esl, :].bitcast(F32R))
        nc.sync.dma_start(hs0_sb[e][:], hs[esl, 0:SBW].bitcast(F32R))
    for e in range(NE):
        nc.sync.dma_start(
            hs123_sb[e][:], hs[128 * e : 128 * (e + 1), SBW:S].bitcast(F32R)
        )
    nc.sync.dma_start(bq_sb[:], bq.rearrange("(m p) -> p m", p=128))
    nc.sync.dma_start(bk_sb[:], bk.rearrange("(m p) -> p m", p=128))
    nc.sync.dma_start(
        bv_bc[:], bass.AP(tensor=bv.tensor, offset=bv.offset, ap=[[0, 128], [1, HD]])
    )
    nc.sync.dma_start(bo_sb[:], bo2.rearrange("(m p) -> p m", p=128))
    for f in range(NM):
        nc.sync.dma_start(woT_sb[f][:], woT[128 * f : 128 * (f + 1), :].bitcast(F32R))

    # v_aug init on DVE (runs during initial DMA wait)
    for t in range(NT):
        nc.vector.memset(v_aug[t][:, :, 0:32], 0.0)
        nc.vector.memset(v_aug[t][:, :, 0:1], 1.0)

    # --- pools ---
    pssc = ctx.enter_context(tc.tile_pool(name="pssc", bufs=2, space="PSUM"))
    psctx = ctx.enter_context(tc.tile_pool(name="psctx", bufs=1, space="PSUM"))
    ps3 = ctx.enter_context(tc.tile_pool(name="ps3", bufs=2, space="PSUM"))
    expp = ctx.enter_context(tc.tile_pool(name="expp", bufs=3))
    smp = ctx.enter_context(tc.tile_pool(name="smp", bufs=1))
    outp = ctx.enter_context(tc.tile_pool(name="outp", bufs=3))

    # --- projection chunks as (steps, finish) for pairwise interleaving of
    # independent PSUM accumulation chains (hides the per-link serialization)
    def kq_chunk(kind, m, s):
        off = WQO if kind == "q" else WKO
        msl = slice(off + 128 * m, off + 128 * (m + 1))
        pp = ps3.tile([128, SBW], F32, tag="o")
        steps = [
            (lambda e=e: nc.tensor.matmul(
                pp[:], wqkv_sb[e][:, msl], hs_at(e, s),
                start=(e == 0), stop=(e == NE - 1),
            ))
            for e in range(NE)
        ]

        def finish():
            if kind == "q":
                nc.vector.tensor_scalar_add(
                    out=qT_sb[m][s][:], in0=pp[:], scalar1=bq_sb[:, m : m + 1]
                )
            else:
                nc.vector.tensor_scalar_add(
                    out=kT_sb[m][:, SBW * s : SBW * (s + 1)], in0=pp[:],
                    scalar1=bk_sb[:, m : m + 1],
                )

        return steps, finish

    def v_chunk(t):
        pp = ps3.tile([128, SBW], F32, tag="o")
        steps = [
            (lambda e=e: nc.tensor.matmul(
                pp[:, 0:HD], hs_tile_at(e, t),
                wqkv_sb[e][:, WVO : WVO + HD],
                start=(e == 0), stop=(e == NE - 1),
            ))
            for e in range(NE)
        ]

        def finish():
            nc.vector.tensor_add(
                out=v_aug[t][:, :, 32 : 32 + D],
                in0=pp[:, 0:HD].rearrange("p (h d) -> p h d", h=NH),
                in1=bv_bc[:].rearrange("p (h d) -> p h d", h=NH),
            )

        return steps, finish

    def out_chunk(s, et):
        esl = slice(128 * et, 128 * (et + 1))
        pp = ps3.tile([128, SBW], F32, tag="o")
        steps = [
            (lambda f=f: nc.tensor.matmul(
                pp[:], woT_sb[f][:, esl], ctxT_sb[f][s][:],
                start=(f == 0), stop=(f == NM - 1),
            ))
            for f in range(NM)
        ]

        def finish():
            ob = outp.tile([128, SBW], F32, tag="ob")
            nc.vector.tensor_scalar_add(
                out=ob[:], in0=pp[:], scalar1=bo_sb[:, et : et + 1]
            )
            nc.sync.dma_start(outT[esl, SBW * s : SBW * (s + 1)], ob[:])

        return steps, finish

    def emit_group(*chunks):
        built = [c() if callable(c) else c for c in chunks]
        n = max(len(st) for st, _ in built)
        for i in range(n):
            for st, _ in built:
                if i < len(st):
                    st[i]()
        for _, fin in built:
            fin()

    ITEMS = [(s, p) for s in range(NS) for p in range(NM)]
    ctx_ps = {}

    def emit_sc_exp(i, g):
        s, p = ITEMS[i]
        tsl = slice(128 * g, 128 * (g + 1))
        sc = pssc.tile([128, 2 * SBW], F32, tag="sc")
        nc.tensor.matmul(
            sc[:, 0:SBW], kT_sb[p][0:D, tsl], qT_sb[p][s][0:D, :],
            start=True, stop=True,
        )
        nc.tensor.matmul(
            sc[:, SBW : 2 * SBW], kT_sb[p][D:128, tsl], qT_sb[p][s][D:128, :],
            start=True, stop=True,
        )
        ex = expp.tile([128, 2 * SBW], BF16, tag="ex")
        nc.scalar.activation(ex[:], sc[:], AF.Exp)
        return ex

    def emit_ctx(i, g, ex):
        s, p = ITEMS[i]
        if g == 0:
            cpA = psctx.tile([128, SBW], F32, tag="cA", name="cpA")
            cpB = psctx.tile([128, SBW], F32, tag="cB", name="cpB")
            ctx_ps[i] = (cpA, cpB)
        cpA, cpB = ctx_ps[i]
        st, sp = (g == 0), (g == NT - 1)
        nc.tensor.matmul(
            cpA[0:VW, :], v_aug[g][:, 2 * p, :], ex[:, 0:SBW], start=st, stop=sp
        )
        nc.tensor.matmul(
            cpB[0:VW, :], v_aug[g][:, 2 * p + 1, :], ex[:, SBW : 2 * SBW],
            start=st, stop=sp,
        )

    def normalize(i):
        s, p = ITEMS[i]
        cpA, cpB = ctx_ps.pop(i)
        for h, cp in ((2 * p, cpA), (2 * p + 1, cpB)):
            o = D * (h % 2)
            tg = "A" if h % 2 == 0 else "B"
            cu = smp.tile([VW, SBW], F32, tag=f"cu{tg}")
            nc.vector.tensor_copy(cu[:], cp[0:VW, :])
            rec = smp.tile([1, SBW], F32, tag=f"rec{tg}")
            nc.vector.reciprocal_approx_fast(out=rec[0:1, :], in_=cu[0:1, :])
            bcs = smp.tile([VW, SBW], F32, tag=f"bcs{tg}")
            nc.gpsimd.partition_broadcast(bcs[:], rec[0:1, :])
            for q in range(2):  # 32-partition chunks (partition shift rules)
                nc.vector.tensor_mul(
                    out=ctxT_sb[p][s][o + 32 * q : o + 32 * (q + 1), :],
                    in0=cu[32 + 32 * q : 64 + 32 * q, :],
                    in1=bcs[32 + 32 * q : 64 + 32 * q, :],
                )

    # --- weave schedule: extras[(item, g)] emitted after sc(g), before the
    # lag-2 ctx drain. Every chunk is placed just ahead of its first consumer
    # so exp starts ~15us in and the PE stays saturated throughout. ---
    extras = {}

    def put(i, g, *specs):
        extras.setdefault((i, g), []).extend(specs)

    # item 0: v tiles (ctx consumes v[t] at unit t+2), k(0, b) ahead of
    # sc unit 4b, then k(1,0)/q(1,0) for item 1
    put(0, 0, lambda: v_chunk(0), lambda: v_chunk(1))
    put(0, 2, lambda: v_chunk(2), lambda: kq_chunk("k", 0, 1))
    put(0, 3, lambda: v_chunk(3))
    put(0, 4, lambda: v_chunk(4), lambda: v_chunk(5))
    put(0, 6, lambda: v_chunk(6), lambda: kq_chunk("k", 0, 2))
    put(0, 7, lambda: v_chunk(7))
    put(0, 8, lambda: v_chunk(8), lambda: v_chunk(9))
    put(0, 10, lambda: v_chunk(10), lambda: kq_chunk("k", 0, 3))
    put(0, 11, lambda: v_chunk(11))
    put(0, 12, lambda: v_chunk(12), lambda: v_chunk(13))
    put(0, 14, lambda: v_chunk(14), lambda: v_chunk(15))
    put(0, 15, lambda: kq_chunk("k", 1, 0), lambda: kq_chunk("q", 1, 0))
    # item 1: k(1, 1..3), then k(2,0)/q(2,0) for item 2
    put(1, 2, lambda: kq_chunk("k", 1, 1))
    put(1, 6, lambda: kq_chunk("k", 1, 2))
    put(1, 10, lambda: kq_chunk("k", 1, 3))
    put(1, 15, lambda: kq_chunk("k", 2, 0), lambda: kq_chunk("q", 2, 0))
    # item 2: k(2, 1..3)
    put(2, 2, lambda: kq_chunk("k", 2, 1))
    put(2, 6, lambda: kq_chunk("k", 2, 2))
    put(2, 10, lambda: kq_chunk("k", 2, 3))
    # q(*, s+1) during item 3s+2
    for s in range(NS - 1):
        for m in range(NM):
            put(3 * s + 2, 3 + 4 * m, lambda m=m, s=s: kq_chunk("q", m, s + 1))
    # out-proj(s-1) pairs during item 3s (normalize(3s-1) lands at unit 1)
    for s in range(1, NS):
        for et in range(0, NE, 2):
            put(
                3 * s, 2 + 2 * et,
                lambda s=s, et=et: out_chunk(s - 1, et),
                lambda s=s, et=et: out_chunk(s - 1, et + 1),
            )

    # --- minimal prefix: k chunk (m=0, block 0) + q chunk (m=0, s0) ---
    emit_group(lambda: kq_chunk("k", 0, 0), lambda: kq_chunk("q", 0, 0))

    # --- fused attention stream with 2-group exp lag across boundaries ---
    from collections import deque

    pend = deque()

    def drain_one():
        pi, pg, pex = pend.popleft()
        emit_ctx(pi, pg, pex)
        if pg == NT - 1:
            normalize(pi)

    for i in range(len(ITEMS)):
        for g in range(NT):
            ex = emit_sc_exp(i, g)
            specs = extras.get((i, g))
            if specs:
                emit_group(*specs)
            if len(pend) == 2:
                drain_one()
            pend.append((i, g, ex))
    while pend:
        drain_one()
    for et in range(0, NE, 2):
        emit_group(
            lambda et=et: out_chunk(NS - 1, et),
            lambda et=et: out_chunk(NS - 1, et + 1),
        )


def build_nc():
    nc = bacc.Bacc("TRN2", target_bir_lowering=False, debug=False)
    hs = nc.dram_tensor("hs", [E, S], F32, kind="ExternalInput")
    wqkv = nc.dram_tensor("wqkv", [E, 3 * HD], F32, kind="ExternalInput")
    bq = nc.dram_tensor("bq", [HD], F32, kind="ExternalInput")
    bk = nc.dram_tensor("bk", [HD], F32, kind="ExternalInput")
    bv = nc.dram_tensor("bv", [HD], F32, kind="ExternalInput")
    woT = nc.dram_tensor("woT", [HD, E], F32, kind="ExternalInput")
    bo2 = nc.dram_tensor("bo2", [E], F32, kind="ExternalInput")
    outT = nc.dram_tensor("outT", [E, S], F32, kind="ExternalOutput")

    with tile.TileContext(nc) as tc:
        mha_tile(
            tc,
            hs[:, :], wqkv[:, :],
            bq[:], bk[:], bv[:],
            woT[:, :], bo2[:], outT[:, :],
        )
    nc.compile()
    return nc


def make_core_inputs(inputs: dict) -> list[dict]:
    """Full inputs -> per-core input maps (core c: batch c//2, head-group c%2)."""
    hsf = np.ascontiguousarray(np.asarray(inputs["hidden_state"], dtype=np.float32))
    Wq = np.asarray(inputs["Wq"], dtype=np.float32)
    Wk = np.asarray(inputs["Wk"], dtype=np.float32)
    Wv = np.asarray(inputs["Wv"], dtype=np.float32)
    Wo = np.asarray(inputs["Wo"], dtype=np.float32)
    bq = np.asarray(inputs["bq"], dtype=np.float32)
    bk = np.asarray(inputs["bk"], dtype=np.float32)
    bv = np.asarray(inputs["bv"], dtype=np.float32)
    bo = np.asarray(inputs["bo"], dtype=np.float32)

    maps = []
    for c in range(8):
        b, g = c // 2, c % 2
        hsl = slice(NH * g, NH * (g + 1))
        fsl = slice(HD * g, HD * (g + 1))
        wq_c = Wq[hsl].transpose(1, 0, 2).reshape(E, HD)
        wk_c = Wk[hsl].transpose(1, 0, 2).reshape(E, HD)
        wv_c = Wv[hsl].transpose(1, 0, 2).reshape(E, HD)
        maps.append(
            {
                "hs": np.ascontiguousarray(hsf[b]),
                "wqkv": np.ascontiguousarray(
                    np.concatenate([wq_c, wk_c, wv_c], axis=1)
                ),
                "bq": np.ascontiguousarray(bq[hsl].reshape(HD)),
                "bk": np.ascontiguousarray(bk[hsl].reshape(HD)),
                "bv": np.ascontiguousarray(bv[hsl].reshape(HD)),
                "woT": np.ascontiguousarray(Wo[:, fsl].T),
                "bo2": np.ascontiguousarray(bo / 2.0),
            }
        )
    return maps


def combine_outputs(core_outs: list) -> np.ndarray:
    """Per-core outT partials -> full [B, E, S] output."""
    return np.stack(
        [core_outs[2 * b]["outT"] + core_outs[2 * b + 1]["outT"] for b in range(B)]
    ).astype(np.float32)


from concourse.bass_utils import run_bass_kernel_spmd

N_CORES = 8
_NC_CACHE = None


def _get_nc():
    global _NC_CACHE
    if _NC_CACHE is None:
        _NC_CACHE = build_nc()
    return _NC_CACHE


def kernel(**inputs) -> np.ndarray:
    """Full-input entry point: shard across 8 cores, run, unshard."""
    maps = make_core_inputs(inputs)
    nc = _get_nc()
    res = run_bass_kernel_spmd(nc, maps, core_ids=list(range(N_CORES)))
    outs = res.results
    return np.stack(
        [outs[2 * b]["outT"] + outs[2 * b + 1]["outT"] for b in range(B)]
    ).astype(np.float32)


# revision 16
# speedup vs baseline: 1.8205x; 1.0321x over previous
"""Multi-head attention Bass/Tile kernel for TRN2, 8-core SPMD.

Sharding: core c handles batch b = c//2 and head-group g = c%2 (6 of 12 heads).
Each core computes its 6 heads end-to-end plus a partial output projection
(over its 384 of 768 ctx dims); the host sums the two partials per batch.

v6 design notes (all calibrated against perfetto traces):
- ScalarE exp is a ~200us serial floor; PE (all matmuls) is ~260us busy.
  The kernel is one fused PE stream ordered so exp starts ~15us in and both
  engines stay saturated: minimal prefix (k-chunk m=0/block0 + q m=0/s0),
  then a flat (s-block, head-pair) attention loop with all other projections
  woven in as "extras" at units where their outputs are first needed.
- 2-group exp pipeline lag: ctx(g-2) is emitted after sc(g), so the PE never
  waits on the exp latency (PE p-state: any stall drops the tensor clock
  2-3.7x; keeping it saturated is worth more than any local reordering).
- DMA queues are descriptor-bound (~78ns per row-descriptor): Wq|Wk|Wv are
  packed host-side into one [E, 1152] tensor (4.5KB rows), hs col-blocks 1-3
  load as [128, 1536] tiles (6KB rows) — inputs land by ~15us.
- PSUM accumulation chains serialize ~50-90ns per link; independent chains
  are emitted pairwise interleaved to hide it.
- Normalize: reciprocal_approx_fast (single DVE op, ~18 bits) + gpsimd
  partition_broadcast; exp/v_aug in bf16 (same PE rate, half the SBUF).
"""

from contextlib import ExitStack

import numpy as np

import concourse.bass as bass
import concourse.tile as tile
from concourse import bacc, mybir
from concourse._compat import with_exitstack

F32R = mybir.dt.float32r
F32 = mybir.dt.float32
BF16 = mybir.dt.bfloat16
AF = mybir.ActivationFunctionType

B, E, S, H, D = 4, 768, 2048, 12, 64
NH = 6          # heads per core
HD = NH * D     # 384 head-dims per core
NE = E // 128   # 6 e-chunks
NM = HD // 128  # 3 m-chunks (2 heads each)
NT = S // 128   # 16 t-tiles
SBW = 512       # s-block width
NS = S // SBW   # 4 s-blocks
VW = 96         # v_aug width: col 0 = ones (denominator), 32..95 = v-dims
WQO, WKO, WVO = 0, HD, 2 * HD  # column offsets in packed wqkv


@with_exitstack
def mha_tile(ctx: ExitStack, tc, hs, wqkv, bq, bk, bv, woT, bo2, outT):
    nc = tc.nc

    persist = ctx.enter_context(tc.tile_pool(name="persist", bufs=1))

    # --- persistent tiles ---
    wqkv_sb = [persist.tile([128, 3 * HD], F32R, name=f"wqkv{e}") for e in range(NE)]
    woT_sb = [persist.tile([128, E], F32R, name=f"wo{f}") for f in range(NM)]
    hs0_sb = [persist.tile([128, SBW], F32R, name=f"hs0_{e}") for e in range(NE)]
    hs123_sb = [persist.tile([128, 3 * SBW], F32R, name=f"hs123_{e}") for e in range(NE)]
    bq_sb = persist.tile([128, NM], F32, name="bq")
    bk_sb = persist.tile([128, NM], F32, name="bk")
    bv_bc = persist.tile([128, HD], F32, name="bv")
    bo_sb = persist.tile([128, NE], F32, name="bo")

    kT_sb = [persist.tile([128, S], F32R, name=f"kT{m}") for m in range(NM)]
    qT_sb = [
        [persist.tile([128, SBW], F32R, name=f"qT{m}_{s}") for s in range(NS)]
        for m in range(NM)
    ]
    ctxT_sb = [
        [persist.tile([128, SBW], F32R, name=f"ctxT{m}_{s}") for s in range(NS)]
        for m in range(NM)
    ]
    v_aug = [persist.tile([128, NH, VW], BF16, name=f"vaug{t}") for t in range(NT)]

    def hs_at(e, s):  # moving [128, 512] for k/q of s-block s
        return hs0_sb[e][:] if s == 0 else hs123_sb[e][:, SBW * (s - 1) : SBW * s]

    def hs_tile_at(e, t):  # stationary [128, 128] for v of t-tile t
        if t < 4:
            return hs0_sb[e][:, 128 * (t % 4) : 128 * (t % 4 + 1)]
        o = 128 * (t - 4)
        return hs123_sb[e][:, o : o + 128]

    # --- DMA issue order = need order (biases first: they gate the PSUM
    # evacuations of the very first projection chunks) ---
    nc.sync.dma_start(bq_sb[:], bq.rearrange("(m p) -> p m", p=128))
    nc.sync.dma_start(bk_sb[:], bk.rearrange("(m p) -> p m", p=128))
    nc.sync.dma_start(
        bv_bc[:], bass.AP(tensor=bv.tensor, offset=bv.offset, ap=[[0, 128], [1, HD]])
    )
    nc.sync.dma_start(bo_sb[:], bo2.rearrange("(m p) -> p m", p=128))
    for e in range(NE):
        esl = slice(128 * e, 128 * (e + 1))
        nc.sync.dma_start(wqkv_sb[e][:], wqkv[esl, :].bitcast(F32R))
        nc.sync.dma_start(hs0_sb[e][:], hs[esl, 0:SBW].bitcast(F32R))
    for e in range(NE):
        nc.sync.dma_start(
            hs123_sb[e][:], hs[128 * e : 128 * (e + 1), SBW:S].bitcast(F32R)
        )
    for f in range(NM):
        nc.sync.dma_start(woT_sb[f][:], woT[128 * f : 128 * (f + 1), :].bitcast(F32R))

    # v_aug init on DVE (runs during initial DMA wait)
    for t in range(NT):
        nc.vector.memset(v_aug[t][:, :, 0:32], 0.0)
        nc.vector.memset(v_aug[t][:, :, 0:1], 1.0)

    # --- pools ---
    pssc = ctx.enter_context(tc.tile_pool(name="pssc", bufs=2, space="PSUM"))
    psctx = ctx.enter_context(tc.tile_pool(name="psctx", bufs=1, space="PSUM"))
    ps3 = ctx.enter_context(tc.tile_pool(name="ps3", bufs=2, space="PSUM"))
    expp = ctx.enter_context(tc.tile_pool(name="expp", bufs=3))
    smp = ctx.enter_context(tc.tile_pool(name="smp", bufs=1))
    outp = ctx.enter_context(tc.tile_pool(name="outp", bufs=3))

    # --- projection chunks as (steps, finish) for pairwise interleaving of
    # independent PSUM accumulation chains (hides the per-link serialization)
    def kq_chunk(kind, m, s):
        off = WQO if kind == "q" else WKO
        msl = slice(off + 128 * m, off + 128 * (m + 1))
        pp = ps3.tile([128, SBW], F32, tag="o")
        steps = [
            (lambda e=e: nc.tensor.matmul(
                pp[:], wqkv_sb[e][:, msl], hs_at(e, s),
                start=(e == 0), stop=(e == NE - 1),
            ))
            for e in range(NE)
        ]

        def finish():
            if kind == "q":
                nc.vector.tensor_scalar_add(
                    out=qT_sb[m][s][:], in0=pp[:], scalar1=bq_sb[:, m : m + 1]
                )
            else:
                nc.vector.tensor_scalar_add(
                    out=kT_sb[m][:, SBW * s : SBW * (s + 1)], in0=pp[:],
                    scalar1=bk_sb[:, m : m + 1],
                )

        return steps, finish

    def v_chunk(t):
        pp = ps3.tile([128, SBW], F32, tag="o")
        steps = [
            (lambda e=e: nc.tensor.matmul(
                pp[:, 0:HD], hs_tile_at(e, t),
                wqkv_sb[e][:, WVO : WVO + HD],
                start=(e == 0), stop=(e == NE - 1),
            ))
            for e in range(NE)
        ]

        def finish():
            nc.vector.tensor_add(
                out=v_aug[t][:, :, 32 : 32 + D],
                in0=pp[:, 0:HD].rearrange("p (h d) -> p h d", h=NH),
                in1=bv_bc[:].rearrange("p (h d) -> p h d", h=NH),
            )

        return steps, finish

    def out_chunk(s, et):
        esl = slice(128 * et, 128 * (et + 1))
        pp = ps3.tile([128, SBW], F32, tag="o")
        steps = [
            (lambda f=f: nc.tensor.matmul(
                pp[:], woT_sb[f][:, esl], ctxT_sb[f][s][:],
                start=(f == 0), stop=(f == NM - 1),
            ))
            for f in range(NM)
        ]

        def finish():
            ob = outp.tile([128, SBW], F32, tag="ob")
            nc.vector.tensor_scalar_add(
                out=ob[:], in0=pp[:], scalar1=bo_sb[:, et : et + 1]
            )
            nc.sync.dma_start(outT[esl, SBW * s : SBW * (s + 1)], ob[:])

        return steps, finish

    def emit_group(*chunks):
        built = [c() if callable(c) else c for c in chunks]
        n = max(len(st) for st, _ in built)
        for i in range(n):
            for st, _ in built:
                if i < len(st):
                    st[i]()
        for _, fin in built:
            fin()

    ITEMS = [(s, p) for s in range(NS) for p in range(NM)]
    ctx_ps = {}

    def emit_sc_exp(i, g):
        s, p = ITEMS[i]
        tsl = slice(128 * g, 128 * (g + 1))
        sc = pssc.tile([128, 2 * SBW], F32, tag="sc")
        nc.tensor.matmul(
            sc[:, 0:SBW], kT_sb[p][0:D, tsl], qT_sb[p][s][0:D, :],
            start=True, stop=True,
        )
        nc.tensor.matmul(
            sc[:, SBW : 2 * SBW], kT_sb[p][D:128, tsl], qT_sb[p][s][D:128, :],
            start=True, stop=True,
        )
        ex = expp.tile([128, 2 * SBW], BF16, tag="ex")
        nc.scalar.activation(ex[:], sc[:], AF.Exp)
        return ex

    def emit_ctx(i, g, ex):
        s, p = ITEMS[i]
        if g == 0:
            cpA = psctx.tile([128, SBW], F32, tag="cA", name="cpA")
            cpB = psctx.tile([128, SBW], F32, tag="cB", name="cpB")
            ctx_ps[i] = (cpA, cpB)
        cpA, cpB = ctx_ps[i]
        st, sp = (g == 0), (g == NT - 1)
        nc.tensor.matmul(
            cpA[0:VW, :], v_aug[g][:, 2 * p, :], ex[:, 0:SBW], start=st, stop=sp
        )
        nc.tensor.matmul(
            cpB[0:VW, :], v_aug[g][:, 2 * p + 1, :], ex[:, SBW : 2 * SBW],
            start=st, stop=sp,
        )

    def normalize(i):
        s, p = ITEMS[i]
        cpA, cpB = ctx_ps.pop(i)
        for h, cp in ((2 * p, cpA), (2 * p + 1, cpB)):
            o = D * (h % 2)
            tg = "A" if h % 2 == 0 else "B"
            cu = smp.tile([VW, SBW], F32, tag=f"cu{tg}")
            nc.vector.tensor_copy(cu[:], cp[0:VW, :])
            rec = smp.tile([1, SBW], F32, tag=f"rec{tg}")
            nc.vector.reciprocal_approx_fast(out=rec[0:1, :], in_=cu[0:1, :])
            bcs = smp.tile([VW, SBW], F32, tag=f"bcs{tg}")
            nc.gpsimd.partition_broadcast(bcs[:], rec[0:1, :])
            for q in range(2):  # 32-partition chunks (partition shift rules)
                nc.vector.tensor_mul(
                    out=ctxT_sb[p][s][o + 32 * q : o + 32 * (q + 1), :],
                    in0=cu[32 + 32 * q : 64 + 32 * q, :],
                    in1=bcs[32 + 32 * q : 64 + 32 * q, :],
                )

    # --- weave schedule: extras[(item, g)] emitted after sc(g), before the
    # lag-2 ctx drain. Every chunk is placed just ahead of its first consumer
    # so exp starts ~15us in and the PE stays saturated throughout. ---
    extras = {}

    def put(i, g, *specs):
        extras.setdefault((i, g), []).extend(specs)

    # item 0: v tiles (ctx consumes v[t] at unit t+2), k(0, b) ahead of
    # sc unit 4b, then k(1,0)/q(1,0) for item 1
    put(0, 0, lambda: v_chunk(0), lambda: v_chunk(1))
    put(0, 2, lambda: v_chunk(2), lambda: kq_chunk("k", 0, 1))
    put(0, 3, lambda: v_chunk(3))
    put(0, 4, lambda: v_chunk(4), lambda: v_chunk(5))
    put(0, 6, lambda: v_chunk(6), lambda: kq_chunk("k", 0, 2))
    put(0, 7, lambda: v_chunk(7))
    put(0, 8, lambda: v_chunk(8), lambda: v_chunk(9))
    put(0, 10, lambda: v_chunk(10), lambda: kq_chunk("k", 0, 3))
    put(0, 11, lambda: v_chunk(11))
    put(0, 12, lambda: v_chunk(12), lambda: v_chunk(13))
    put(0, 14, lambda: v_chunk(14), lambda: v_chunk(15))
    put(0, 15, lambda: kq_chunk("k", 1, 0), lambda: kq_chunk("q", 1, 0))
    # item 1: k(1, 1..3), then k(2,0)/q(2,0) for item 2
    put(1, 2, lambda: kq_chunk("k", 1, 1))
    put(1, 6, lambda: kq_chunk("k", 1, 2))
    put(1, 10, lambda: kq_chunk("k", 1, 3))
    put(1, 15, lambda: kq_chunk("k", 2, 0), lambda: kq_chunk("q", 2, 0))
    # item 2: k(2, 1..3)
    put(2, 2, lambda: kq_chunk("k", 2, 1))
    put(2, 6, lambda: kq_chunk("k", 2, 2))
    put(2, 10, lambda: kq_chunk("k", 2, 3))
    # q(*, s+1) during item 3s+2
    for s in range(NS - 1):
        for m in range(NM):
            put(3 * s + 2, 3 + 4 * m, lambda m=m, s=s: kq_chunk("q", m, s + 1))
    # out-proj(s-1) pairs during item 3s: normalize(3s-1) lands at unit 1 and
    # its DVE burst runs ~units 1-5, so start at unit 6 to keep the ps3
    # evacuations (also DVE) from stalling the rotation
    for s in range(1, NS):
        for et in range(0, NE, 2):
            put(
                3 * s, 6 + 2 * et,
                lambda s=s, et=et: out_chunk(s - 1, et),
                lambda s=s, et=et: out_chunk(s - 1, et + 1),
            )

    # --- minimal prefix: k chunk (m=0, block 0) + q chunk (m=0, s0) ---
    emit_group(lambda: kq_chunk("k", 0, 0), lambda: kq_chunk("q", 0, 0))

    # --- fused attention stream with 2-group exp lag across boundaries ---
    from collections import deque

    pend = deque()

    def drain_one():
        pi, pg, pex = pend.popleft()
        emit_ctx(pi, pg, pex)
        if pg == NT - 1:
            normalize(pi)

    for i in range(len(ITEMS)):
        for g in range(NT):
            ex = emit_sc_exp(i, g)
            specs = extras.get((i, g))
            if specs:
                emit_group(*specs)
            if len(pend) == 2:
                drain_one()
            pend.append((i, g, ex))
    while pend:
        drain_one()
    for et in range(0, NE, 2):
        emit_group(
            lambda et=et: out_chunk(NS - 1, et),
            lambda et=et: out_chunk(NS - 1, et + 1),
        )


def build_nc():
    nc = bacc.Bacc("TRN2", target_bir_lowering=False, debug=False)
    hs = nc.dram_tensor("hs", [E, S], F32, kind="ExternalInput")
    wqkv = nc.dram_tensor("wqkv", [E, 3 * HD], F32, kind="ExternalInput")
    bq = nc.dram_tensor("bq", [HD], F32, kind="ExternalInput")
    bk = nc.dram_tensor("bk", [HD], F32, kind="ExternalInput")
    bv = nc.dram_tensor("bv", [HD], F32, kind="ExternalInput")
    woT = nc.dram_tensor("woT", [HD, E], F32, kind="ExternalInput")
    bo2 = nc.dram_tensor("bo2", [E], F32, kind="ExternalInput")
    outT = nc.dram_tensor("outT", [E, S], F32, kind="ExternalOutput")

    with tile.TileContext(nc) as tc:
        mha_tile(
            tc,
            hs[:, :], wqkv[:, :],
            bq[:], bk[:], bv[:],
            woT[:, :], bo2[:], outT[:, :],
        )
    nc.compile()
    return nc


def make_core_inputs(inputs: dict) -> list[dict]:
    """Full inputs -> per-core input maps (core c: batch c//2, head-group c%2)."""
    hsf = np.ascontiguousarray(np.asarray(inputs["hidden_state"], dtype=np.float32))
    Wq = np.asarray(inputs["Wq"], dtype=np.float32)
    Wk = np.asarray(inputs["Wk"], dtype=np.float32)
    Wv = np.asarray(inputs["Wv"], dtype=np.float32)
    Wo = np.asarray(inputs["Wo"], dtype=np.float32)
    bq = np.asarray(inputs["bq"], dtype=np.float32)
    bk = np.asarray(inputs["bk"], dtype=np.float32)
    bv = np.asarray(inputs["bv"], dtype=np.float32)
    bo = np.asarray(inputs["bo"], dtype=np.float32)

    maps = []
    for c in range(8):
        b, g = c // 2, c % 2
        hsl = slice(NH * g, NH * (g + 1))
        fsl = slice(HD * g, HD * (g + 1))
        wq_c = Wq[hsl].transpose(1, 0, 2).reshape(E, HD)
        wk_c = Wk[hsl].transpose(1, 0, 2).reshape(E, HD)
        wv_c = Wv[hsl].transpose(1, 0, 2).reshape(E, HD)
        maps.append(
            {
                "hs": np.ascontiguousarray(hsf[b]),
                "wqkv": np.ascontiguousarray(
                    np.concatenate([wq_c, wk_c, wv_c], axis=1)
                ),
                "bq": np.ascontiguousarray(bq[hsl].reshape(HD)),
                "bk": np.ascontiguousarray(bk[hsl].reshape(HD)),
                "bv": np.ascontiguousarray(bv[hsl].reshape(HD)),
                "woT": np.ascontiguousarray(Wo[:, fsl].T),
                "bo2": np.ascontiguousarray(bo / 2.0),
            }
        )
    return maps


def combine_outputs(core_outs: list) -> np.ndarray:
    """Per-core outT partials -> full [B, E, S] output."""
    return np.stack(
        [core_outs[2 * b]["outT"] + core_outs[2 * b + 1]["outT"] for b in range(B)]
    ).astype(np.float32)


from concourse.bass_utils import run_bass_kernel_spmd

N_CORES = 8
_NC_CACHE = None


def _get_nc():
    global _NC_CACHE
    if _NC_CACHE is None:
        _NC_CACHE = build_nc()
    return _NC_CACHE


def kernel(**inputs) -> np.ndarray:
    """Full-input entry point: shard across 8 cores, run, unshard."""
    maps = make_core_inputs(inputs)
    nc = _get_nc()
    res = run_bass_kernel_spmd(nc, maps, core_ids=list(range(N_CORES)))
    outs = res.results
    return np.stack(
        [outs[2 * b]["outT"] + outs[2 * b + 1]["outT"] for b in range(B)]
    ).astype(np.float32)
